# revision 1
# baseline (speedup 1.0000x reference)
"""Multi-head attention (B=4, S=2048, D=512, H=8, DH=64) on 8 TRN2 NeuronCores.

Sharding: core c handles batch b = c//2 and head-group g = c%2 (4 of the 8
heads).  Each core computes its QKV projection (columns of W_qkv for its
heads), attention for its 4 heads, and a partial output projection
(rows of W_out for its heads).  The host sums the two partials per batch
and adds the bias.

Per-core device layout (compute in bf16, fp32 PSUM accumulation):
  - host passes x[b] pre-transposed (xT [D, S]); the QKV projection then
    needs no on-device transpose: qkT[f, s] = sum_d wqk[d, f] * xT[d, s].
  - qT/kT are stored one head per 128-partition chunk with the unused 64
    partitions zeroed, so every matmul in the kernel runs in the same
    128x128 array mode (no TensorE mode-switch drains) and score matmuls
    contract over K=128 (the zero rows contribute nothing).
  - scores are computed transposed (scoresT [k, q]); exp(scale*s) is fused
    into the PSUM->SBUF copy on the Scalar engine, batched 2 PSUM banks at
    a time to amortize the ACTIVATE fixed overhead.
  - attn @ V uses V in natural [token, dh] layout augmented with a ones
    column: one PSUM accumulation produces outT_unnorm [dh, q] AND the
    softmax denominator row.
  - normalization: reciprocal of the denominator row, DMA-broadcast across
    64 partitions, multiply on the Vector engine.
  - output projection: lhsT = outT [128 (2 heads), 128 q] against the
    matching W_out rows, accumulated over head pairs -> y [q, DO].
  - emission is software-pipelined so the TensorE never starves (a stall
    >3.4us drops the HAM clock gate from 2.4GHz to 1.2GHz): attn@V matmuls
    of head h-1 and the previous tile's output projection are woven
    between score matmul groups as ACT-independent filler.
"""

import sys

for _p in ("/opt/trn_rl_repo", "/root/.axon_site/_ro/trn_rl_repo"):
    if _p not in sys.path:
        sys.path.append(_p)

import ml_dtypes
import numpy as np

import concourse.bass as bass
import concourse.tile as tile
from concourse import bacc, mybir

F32 = mybir.dt.float32
F32R = mybir.dt.float32r
BF16 = mybir.dt.bfloat16
AF = mybir.ActivationFunctionType

# Problem dims (hardcoded per the grading contract).
B, S, D = 4, 2048, 512
H, DH = 8, 64
INNER = H * DH
HL = 4                # heads per core
DO = D                # output dim
QT = 512              # query tile
SCALE = DH ** -0.5

N_CORES = 8
COMPUTE_DT = "bf16"   # "bf16" | "f32r"


def build_nc(S=S, D=D, HL=HL, DH=DH, DO=DO, QT=QT, n_cores=N_CORES,
             compute_dt=COMPUTE_DT):
    KB = S // 128         # k-token blocks
    DC = D // 128         # contraction chunks for the projections
    QKF = 2 * HL * DH     # q+k feature count per core
    MQK = QKF // 128      # qk feature blocks (2 heads each)
    VF = HL * DH          # v feature count per core
    NQT = S // QT         # query tiles
    SG = 2                # exp supergroup: PSUM banks per ACTIVATE
    NG = KB // SG         # score groups per head

    if compute_dt == "bf16":
        CDT = BF16
        in_dt = BF16

        def incast(ap):
            return ap
    else:
        CDT = F32R
        in_dt = F32

        def incast(ap):
            return ap.bitcast(F32R)

    nc = bacc.Bacc(
        "TRN2", target_bir_lowering=False, debug=False, num_devices=n_cores
    )
    xT = nc.dram_tensor("xT", [D, S], in_dt, kind="ExternalInput").ap()
    wqk = nc.dram_tensor("wqk", [D, QKF], in_dt, kind="ExternalInput").ap()
    wv = nc.dram_tensor("wv", [D, VF], in_dt, kind="ExternalInput").ap()
    wo = nc.dram_tensor("wo", [VF, DO], in_dt, kind="ExternalInput").ap()
    y = nc.dram_tensor("y", [S, DO], F32, kind="ExternalOutput").ap()

    with tile.TileContext(nc) as tc:
        with (
            tc.tile_pool(name="weights", bufs=1) as wpool,
            tc.tile_pool(name="big", bufs=1) as big,
        ):
            # ---- loads ----
            wqk_sb = wpool.tile([128, DC, QKF], CDT)
            nc.sync.dma_start(
                out=wqk_sb, in_=incast(wqk.rearrange("(c p) f -> p c f", p=128))
            )
            xT_sb = big.tile([128, DC, S], CDT)
            x_view = xT.rearrange("(c p) s -> c p s", p=128)
            for c in range(DC):
                sl = slice(0, S // 2)
                nc.sync.dma_start(
                    out=xT_sb[:, c, sl], in_=incast(x_view[c][:, sl])
                )
            wv_sb = wpool.tile([128, DC, VF], CDT)
            nc.sync.dma_start(
                out=wv_sb, in_=incast(wv.rearrange("(c p) f -> p c f", p=128))
            )
            wo_sb = wpool.tile([128, HL // 2, DO], CDT)
            nc.sync.dma_start(
                out=wo_sb, in_=incast(wo.rearrange("(c p) d -> p c d", p=128))
            )
            for c in range(DC):
                sl = slice(S // 2, S)
                nc.sync.dma_start(
                    out=xT_sb[:, c, sl], in_=incast(x_view[c][:, sl])
                )

            # ---- phase A: projections ----
            # qkT chunk h    = qT of head h  (real rows (h%2)*64..+64, rest 0)
            # qkT chunk HL+h = kT of head h  (same padding)
            qkT = big.tile([128, 2 * HL, S], CDT)
            nc.vector.memset(qkT, 0.0)
            vaug = big.tile([128, KB, HL, DH + 1], CDT)
            with tc.tile_pool(name="psA", bufs=4, space="PSUM") as psA:
                def qk_block(t):
                    for m in range(MQK):
                        base = HL if m >= MQK // 2 else 0
                        hp = 2 * (m % (MQK // 2))
                        sl = slice(t * 512, (t + 1) * 512)
                        ps = psA.tile([128, 512], F32, tag="qk", name="psqk")
                        for c in range(DC):
                            nc.tensor.matmul(
                                ps,
                                lhsT=wqk_sb[:, c, m * 128:(m + 1) * 128],
                                rhs=xT_sb[:, c, sl],
                                start=(c == 0),
                                stop=(c == DC - 1),
                            )
                        nc.scalar.copy(
                            out=qkT[0:64, base + hp, sl], in_=ps[0:64, :]
                        )
                        nc.vector.tensor_copy(
                            out=qkT[64:128, base + hp + 1, sl],
                            in_=ps[64:128, :],
                        )

                def v_block(t):
                    ps = psA.tile([128, VF], F32, tag="v", name="psv")
                    for c in range(DC):
                        nc.tensor.matmul(
                            ps,
                            lhsT=xT_sb[:, c, t * 128:(t + 1) * 128],
                            rhs=wv_sb[:, c, :],
                            start=(c == 0),
                            stop=(c == DC - 1),
                        )
                    nc.scalar.copy(
                        out=vaug[:, t, :, 0:DH],
                        in_=ps.rearrange("p (h e) -> p h e", h=HL),
                    )

                # V in natural [token, dh] layout, +1 ones column per head.
                ones_col = vaug[:, :, :, DH:DH + 1]
                nc.vector.memset(
                    ones_col.bitcast(F32) if CDT is F32R else ones_col, 1.0
                )
                # token-half-0 work first (its DMAs land first)
                qk_block(0)
                qk_block(1)
                for t in range(KB // 2):
                    v_block(t)
                qk_block(2)
                qk_block(3)
                for t in range(KB // 2, KB):
                    v_block(t)

            # ---- phase B: attention + output projection (woven) ----
            with (
                tc.tile_pool(name="psS", bufs=2, space="PSUM") as psS,
                tc.tile_pool(name="psB2", bufs=4, space="PSUM") as psB2,
                tc.tile_pool(name="attnp", bufs=5) as attnp,
                tc.tile_pool(name="outp", bufs=2) as outp,
                tc.tile_pool(name="smalls", bufs=3) as smalls,
                tc.tile_pool(name="ysbp", bufs=3) as ysbp,
            ):
                # proj work left over from the previous q-tile: closures,
                # each emitting one PSUM accumulation + store.
                pending_proj = []

                def make_proj_units(outT, n):
                    units = []
                    for qb in range(QT // 128):
                        def unit(qb=qb, outT=outT, n=n):
                            yps = psB2.tile([128, DO], F32, tag="bank")
                            for c in range(HL // 2):
                                nc.tensor.matmul(
                                    yps,
                                    lhsT=outT[:, c, qb * 128:(qb + 1) * 128],
                                    rhs=wo_sb[:, c, :],
                                    start=(c == 0),
                                    stop=(c == HL // 2 - 1),
                                    skip_group_check=True,
                                )
                            ysb = ysbp.tile([128, DO], F32, tag="ysb")
                            nc.vector.tensor_copy(out=ysb, in_=yps)
                            nc.sync.dma_start(
                                out=y[n * QT + qb * 128:
                                      n * QT + (qb + 1) * 128, :],
                                in_=ysb,
                            )
                        units.append(unit)
                    return units

                carry = None  # last av chunks + normalize of prev tile's h3

                for n in range(NQT):
                    outT = outp.tile([128, HL // 2, QT], CDT, tag="outT")
                    at = {}
                    avps = {}
                    avk = {h: 0 for h in range(HL)}

                    def score_unit(h, g, n=n, at=at):
                        if g == 0:
                            at[h] = attnp.tile([128, KB, QT], CDT, tag="attnT", name="at")
                        qs = qkT[:, h, n * QT:(n + 1) * QT]
                        ps = psS.tile([128, SG, 512], F32, tag="sc")
                        for i in range(SG):
                            kb = g * SG + i
                            nc.tensor.matmul(
                                ps[:, i, :],
                                lhsT=qkT[:, HL + h, kb * 128:(kb + 1) * 128],
                                rhs=qs,
                                skip_group_check=True,
                            )
                        nc.scalar.activation(
                            out=at[h][:, g * SG:(g + 1) * SG, :], in_=ps,
                            func=AF.Exp, scale=SCALE,
                        )

                    def normalize(h, outT=outT, avps=avps):
                        ps = avps[h]
                        rd = smalls.tile([DH + 1, QT], F32, tag="rd")
                        nc.vector.reciprocal(
                            rd[DH:DH + 1, :], ps[DH:DH + 1, :]
                        )
                        rd0 = smalls.tile([1, QT], F32, tag="rd0")
                        nc.sync.dma_start(out=rd0, in_=rd[DH:DH + 1, :])
                        rb = smalls.tile([64, QT], F32, tag="rb")
                        nc.gpsimd.partition_broadcast(rb, rd0, channels=64)
                        if h % 2 == 0:
                            nc.vector.tensor_mul(
                                outT[0:64, h // 2, :], ps[0:DH, :], rb
                            )
                        else:
                            ot = smalls.tile([64, QT], CDT, tag="ot")
                            nc.vector.tensor_mul(ot, ps[0:DH, :], rb)
                            nc.sync.dma_start(
                                out=outT[64:128, h // 2, :], in_=ot
                            )

                    def av_mms(h, cnt, at=at, avps=avps, avk=avk,
                               normalize=normalize):
                        for _ in range(cnt):
                            kb = avk[h]
                            avk[h] = kb + 1
                            if kb == 0:
                                avps[h] = psB2.tile(
                                    [DH + 1, QT], F32, tag="bank", name="avp"
                                )
                            nc.tensor.matmul(
                                avps[h],
                                lhsT=vaug[:, kb, h, :],
                                rhs=at[h][:, kb, :],
                                start=(kb == 0),
                                stop=(kb == KB - 1),
                                skip_group_check=True,
                            )
                        if avk[h] == KB:
                            normalize(h)

                    # Weave: head h's attn@V follows its own scores one
                    # group behind; the last two chunks + normalize land on
                    # the next head's (or next tile's) first slot, so the
                    # four normalize chains spread evenly instead of
                    # bunching on the Vector engine.
                    # Head order ends on an even head so the final
                    # normalize chain before the projection has no
                    # partition-shift DMA in it.
                    HEAD_ORDER = (1, 3, 0, 2)
                    for idx, h in enumerate(HEAD_ORDER):
                        for g in range(NG):
                            score_unit(h, g)
                            if g == 0:
                                if idx == 0:
                                    if carry is not None:
                                        carry()
                                        carry = None
                                else:
                                    av_mms(HEAD_ORDER[idx - 1], 2)
                            else:
                                av_mms(h, 2)
                            if idx == 1 and pending_proj:
                                pending_proj.pop(0)()

                    def make_carry(av_mms=av_mms):
                        return lambda: av_mms(2, 2)

                    carry = make_carry()
                    pending_proj = make_proj_units(outT, n)

                if carry is not None:
                    carry()

                for u in pending_proj:
                    u()

    nc.compile()
    return nc


def shard_inputs(x, W_qkv, W_out, compute_dt=COMPUTE_DT):
    """Full inputs -> list of 8 per-core input maps."""
    dt = ml_dtypes.bfloat16 if compute_dt == "bf16" else np.float32
    in_maps = []
    for c in range(N_CORES):
        b, g = divmod(c, 2)
        qcols = W_qkv[:, g * 256:(g + 1) * 256]
        kcols = W_qkv[:, INNER + g * 256:INNER + (g + 1) * 256]
        vcols = W_qkv[:, 2 * INNER + g * 256:2 * INNER + (g + 1) * 256]
        in_maps.append({
            "xT": np.ascontiguousarray(x[b].T).astype(dt),
            "wqk": np.ascontiguousarray(
                np.concatenate([qcols, kcols], axis=1)).astype(dt),
            "wv": np.ascontiguousarray(vcols).astype(dt),
            "wo": np.ascontiguousarray(
                W_out[g * 256:(g + 1) * 256, :]).astype(dt),
        })
    return in_maps


def gather_output(ys, b_out):
    out = np.empty((B, S, DO), np.float32)
    for b in range(B):
        out[b] = ys[2 * b] + ys[2 * b + 1]
        out[b] += b_out
    return out


_NC_CACHE = {}


def _get_nc():
    if "nc" not in _NC_CACHE:
        _NC_CACHE["nc"] = build_nc()
    return _NC_CACHE["nc"]


def kernel(**inputs):
    x = np.asarray(inputs["x"], np.float32)
    W_qkv = np.asarray(inputs["W_qkv"], np.float32)
    W_out = np.asarray(inputs["W_out"], np.float32)
    b_out = np.asarray(inputs["b_out"], np.float32)

    from concourse.bass_utils import run_bass_kernel_spmd

    nc = _get_nc()
    in_maps = shard_inputs(x, W_qkv, W_out)
    res = run_bass_kernel_spmd(nc, in_maps, core_ids=list(range(N_CORES)))
    ys = [r["y"] for r in res.results]
    return gather_output(ys, b_out)



# revision 18
# speedup vs baseline: 1.0301x; 1.0301x over previous
"""Multi-head attention (B=4, S=2048, D=512, H=8, DH=64) on 8 TRN2 NeuronCores.

Sharding: core c handles batch b = c//2 and head-group g = c%2 (4 of the 8
heads).  Each core computes its QKV projection (columns of W_qkv for its
heads), attention for its 4 heads, and a partial output projection
(rows of W_out for its heads).  The host sums the two partials per batch
and adds the bias.

v2 design notes (per-core; compute bf16, fp32 PSUM):
  - The Scalar (ACT) engine is a hard floor: exp over 16.8M attention
    scores at ~1 elem/cycle/partition @1.2GHz ~= 133us.  TensorE stream
    cycles total ~136us.  The schedule keeps both engines continuously
    busy; everything else (copies, normalize, DMAs) is pushed to
    DVE/GpSimd/sync which have slack.
  - q/k are stored unpadded, head-pairs packed across the 128 partitions
    (qk2[:, 0..3] = q01, q23, k01, k23; head h uses partitions
    (h%2)*64..+64).  Score matmuls contract over K=64.  No memset, half
    the copy volume of the padded layout.
  - Input DMAs are split and ordered (k01 weights + first x half first)
    so the first score matmuls issue ~2us in; the QKV projection work
    that is not needed immediately is woven as filler between tile-0
    score groups, so exp starts at ~6us instead of ~22us.
  - attn@V uses V in [token, dh+1] layout (ones column -> softmax
    denominator row in the same PSUM accumulation).  AV matmuls are fed
    from a work queue: each score group slot pops up to `cap` pending
    AV units (oldest head first), which self-balances the one-head-
    behind weave across tile boundaries and drains the backlog fast in
    the last tile.
  - normalize: reciprocal_approx_fast (5x cheaper than reciprocal, 18
    bits), partition-shift DMA + broadcast on the GpSimd queue (keeps
    the sync queue free for input/output DMAs), multiply on DVE.
  - out-projection of tile n runs as filler during tile n+1; tile 3's
    runs at the end (tail ~5us).
"""

import sys

for _p in ("/opt/trn_rl_repo", "/root/.axon_site/_ro/trn_rl_repo"):
    if _p not in sys.path:
        sys.path.append(_p)

import ml_dtypes
import numpy as np

import concourse.bass as bass
import concourse.tile as tile
from concourse import bacc, mybir

F32 = mybir.dt.float32
BF16 = mybir.dt.bfloat16
AF = mybir.ActivationFunctionType

# Problem dims (hardcoded per the grading contract).
B, S, D = 4, 2048, 512
H, DH = 8, 64
INNER = H * DH
HL = 4                # heads per core
DO = D                # output dim
QT = 512              # query tile
SCALE = DH ** -0.5

N_CORES = 8
CDT = BF16
# reciprocal_approx_fast (custom DVE ucode) returns garbage on this
# hardware/runtime (verified: CoreSim-correct, HW-NaN).  Exact reciprocal
# costs 3.3us per call on DVE but DVE has slack.
USE_APPROX_RECIP = False

KB = S // 128          # k-token blocks (16)
DC = D // 128          # contraction chunks for projections (4)
QKF = 2 * HL * DH      # q+k feature count per core (512)
MQK = QKF // 128       # qk feature blocks (4): m0=q01 m1=q23 m2=k01 m3=k23
VF = HL * DH           # v feature count (256)
NQT = S // QT          # query tiles (4)
SG = 2                 # exp supergroup: PSUM banks per ACTIVATE
NG = KB // SG          # score groups per head (8)


def build_nc(n_cores=N_CORES):
    nc = bacc.Bacc(
        "TRN2", target_bir_lowering=False, debug=False, num_devices=n_cores
    )
    xT = nc.dram_tensor("xT", [D, S], CDT, kind="ExternalInput").ap()
    wqk = nc.dram_tensor("wqk", [D, QKF], CDT, kind="ExternalInput").ap()
    wv = nc.dram_tensor("wv", [D, VF], CDT, kind="ExternalInput").ap()
    wo = nc.dram_tensor("wo", [VF, DO], CDT, kind="ExternalInput").ap()
    y = nc.dram_tensor("y", [S, DO], F32, kind="ExternalOutput").ap()

    with tile.TileContext(nc) as tc:
        with (
            tc.tile_pool(name="weights", bufs=1) as wpool,
            tc.tile_pool(name="big", bufs=1) as big,
            tc.tile_pool(name="psS", bufs=2, space="PSUM") as psS,
            tc.tile_pool(name="psB", bufs=2, space="PSUM") as psB,
            tc.tile_pool(name="psAV", bufs=2, space="PSUM") as psAV,
            tc.tile_pool(name="attnp", bufs=6) as attnp,
            tc.tile_pool(name="outp", bufs=2) as outp,
            tc.tile_pool(name="smalls", bufs=3) as smalls,
            tc.tile_pool(name="ysbp", bufs=3) as ysbp,
        ):
            # ---- input DMAs, ordered so k01 + q01/t0 compute starts asap ----
            wqk_sb = wpool.tile([128, DC, QKF], CDT)
            wqk_v = wqk.rearrange("(c p) f -> p c f", p=128)

            def wqk_dma(m):
                nc.sync.dma_start(
                    out=wqk_sb[:, :, m * 128:(m + 1) * 128],
                    in_=wqk_v[:, :, m * 128:(m + 1) * 128],
                )

            xT_sb = big.tile([128, DC, S], CDT)
            x_view = xT.rearrange("(c p) s -> c p s", p=128)

            def x_dma(c, half):
                sl = slice(half * (S // 2), (half + 1) * (S // 2))
                nc.sync.dma_start(out=xT_sb[:, c, sl], in_=x_view[c][:, sl])

            wv_sb = wpool.tile([128, DC, VF], CDT)
            wo_sb = wpool.tile([128, HL // 2, DO], CDT)

            wqk_dma(2)
            for c in range(DC):
                x_dma(c, 0)
            wqk_dma(0)
            nc.sync.dma_start(
                out=wv_sb, in_=wv.rearrange("(c p) f -> p c f", p=128)
            )
            for c in range(DC):
                x_dma(c, 1)
            wqk_dma(3)
            wqk_dma(1)
            nc.sync.dma_start(
                out=wo_sb, in_=wo.rearrange("(c p) d -> p c d", p=128)
            )

            # ---- persistent SBUF tensors ----
            # qkT chunk h    = qT of head h  (real rows (h%2)*64..+64, rest 0)
            # qkT chunk HL+h = kT of head h  (same padding).  The zero
            # padding keeps every matmul at K=128 (no mode-switch drains,
            # FWL stays enabled).
            qkT = big.tile([128, 2 * HL, S], CDT)
            nc.vector.memset(qkT, 0.0)
            vaug = big.tile([128, KB, HL, DH + 1], CDT)
            ones_col = vaug[:, :, :, DH:DH + 1]
            nc.vector.memset(ones_col, 1.0)

            # ---- phase A work units (emitted inline or as phase-B filler) ----
            def qk_unit(m, t):
                # m: wqk feature block (m0=q01 m1=q23 m2=k01 m3=k23);
                # t: 512-token slice
                sl = slice(t * 512, (t + 1) * 512)
                base = HL if m >= 2 else 0
                hp = 2 * (m % 2)
                ps = psB.tile([128, 512], F32, tag="bank", name="psqk")
                for c in range(DC):
                    nc.tensor.matmul(
                        ps,
                        lhsT=wqk_sb[:, c, m * 128:(m + 1) * 128],
                        rhs=xT_sb[:, c, sl],
                        start=(c == 0),
                        stop=(c == DC - 1),
                    )
                nc.vector.tensor_copy(out=qkT[0:64, base + hp, sl],
                                      in_=ps[0:64, :])
                nc.vector.tensor_copy(out=qkT[64:128, base + hp + 1, sl],
                                      in_=ps[64:128, :])

            def v_unit(t):
                # t: 128-token block
                ps = psB.tile([128, 512], F32, tag="bank", name="psv")
                for c in range(DC):
                    nc.tensor.matmul(
                        ps[:, 0:VF],
                        lhsT=xT_sb[:, c, t * 128:(t + 1) * 128],
                        rhs=wv_sb[:, c, :],
                        start=(c == 0),
                        stop=(c == DC - 1),
                    )
                nc.vector.tensor_copy(
                    out=vaug[:, t, :, 0:DH],
                    in_=ps[:, 0:VF].rearrange("p (h e) -> p h e", h=HL),
                )

            # k01 for tokens 0-511 and q01 tile0 up front; everything else
            # becomes filler.
            qk_unit(2, 0)
            qk_unit(0, 0)
            # tile0-idx0 slots take 1 filler each (8), idx1 takes 2 (16),
            # idx2 takes 1 (6).  Placement respects score/AV dependencies
            # (see dependency notes in av_pop/score loop).
            fillers = [
                ("qk", 2, 1), ("v", 0), ("v", 1), ("qk", 2, 2),
                ("v", 2), ("qk", 2, 3), ("v", 3), ("qk", 1, 0),
                ("v", 4), ("v", 5), ("v", 6), ("v", 7),
                ("v", 8), ("v", 9), ("v", 10), ("v", 11),
                ("v", 12), ("v", 13), ("v", 14), ("v", 15),
                ("qk", 3, 0), ("qk", 3, 1), ("qk", 3, 2), ("qk", 3, 3),
                ("qk", 0, 1), ("qk", 1, 1),
                ("qk", 0, 2), ("qk", 1, 2),
                ("qk", 0, 3), ("qk", 1, 3),
            ]

            def pop_filler(k):
                n = 0
                while fillers and n < k:
                    u = fillers.pop(0)
                    if u[0] == "qk":
                        qk_unit(u[1], u[2])
                    else:
                        v_unit(u[1])
                    n += 1

            # ---- phase B: attention + output projection ----
            # AV work queue: entries (h_key, kb, tile_n); emitted oldest
            # first, up to `cap` per score-group slot.
            av_q = []
            avps = {}     # h_key -> psum tile
            outT_of = {}  # h_key -> (outT tile, h)

            def normalize(h_key):
                outT, h = outT_of[h_key]
                ps = avps.pop(h_key)
                rd = smalls.tile([DH + 1, QT], F32, tag="rd")
                if USE_APPROX_RECIP:
                    nc.vector.reciprocal_approx_fast(
                        out=rd[DH:DH + 1, :], in_=ps[DH:DH + 1, :]
                    )
                else:
                    nc.vector.reciprocal(
                        rd[DH:DH + 1, :], ps[DH:DH + 1, :]
                    )
                rd0 = smalls.tile([1, QT], F32, tag="rd0")
                nc.sync.dma_start(out=rd0, in_=rd[DH:DH + 1, :])
                rb = smalls.tile([64, QT], F32, tag="rb")
                nc.gpsimd.partition_broadcast(rb, rd0, channels=64)
                if h % 2 == 0:
                    nc.vector.tensor_mul(
                        outT[0:64, h // 2, :], ps[0:DH, :], rb
                    )
                else:
                    ot = smalls.tile([64, QT], CDT, tag="ot")
                    nc.vector.tensor_mul(ot, ps[0:DH, :], rb)
                    nc.sync.dma_start(out=outT[64:128, h // 2, :], in_=ot)

            def av_pop(cap):
                n = 0
                while av_q and n < cap:
                    h_key, kb, tn = av_q.pop(0)
                    if kb == 0:
                        avps[h_key] = psAV.tile(
                            [DH + 1, QT], F32, tag="avp", name="avp"
                        )
                    _, h = outT_of[h_key]
                    at = at_of[h_key]
                    nc.tensor.matmul(
                        avps[h_key],
                        lhsT=vaug[:, kb, h, :],
                        rhs=at[:, kb, :],
                        start=(kb == 0),
                        stop=(kb == KB - 1),
                        skip_group_check=True,
                    )
                    n += 1
                    if kb == KB - 1:
                        normalize(h_key)

            at_of = {}

            def score_unit(h_key, h, n, g):
                if g == 0:
                    at_of[h_key] = attnp.tile(
                        [128, KB, QT], CDT, tag="attnT", name="at"
                    )
                qs = qkT[:, h, n * QT:(n + 1) * QT]
                ps = psS.tile([128, SG, 512], F32, tag="sc")
                for i in range(SG):
                    kb = g * SG + i
                    nc.tensor.matmul(
                        ps[:, i, :],
                        lhsT=qkT[:, HL + h, kb * 128:(kb + 1) * 128],
                        rhs=qs,
                        skip_group_check=True,
                    )
                nc.scalar.activation(
                    out=at_of[h_key][:, g * SG:(g + 1) * SG, :], in_=ps,
                    func=AF.Exp, scale=SCALE,
                )

            pending_proj = []

            def make_proj_units(outT, n):
                units = []
                for qb in range(QT // 128):
                    def unit(qb=qb, outT=outT, n=n):
                        yps = psB.tile([128, DO], F32, tag="bank", name="yps")
                        for c in range(HL // 2):
                            nc.tensor.matmul(
                                yps,
                                lhsT=outT[:, c, qb * 128:(qb + 1) * 128],
                                rhs=wo_sb[:, c, :],
                                start=(c == 0),
                                stop=(c == HL // 2 - 1),
                                skip_group_check=True,
                            )
                        ysb = ysbp.tile([128, DO], F32, tag="ysb")
                        nc.vector.tensor_copy(out=ysb, in_=yps)
                        nc.sync.dma_start(
                            out=y[n * QT + qb * 128:
                                  n * QT + (qb + 1) * 128, :],
                            in_=ysb,
                        )
                    units.append(unit)
                return units

            for n in range(NQT):
                outT = outp.tile([128, HL // 2, QT], CDT, tag="outT")
                order = (1, 0, 3, 2) if n == 0 else (1, 3, 0, 2)
                for idx, h in enumerate(order):
                    h_key = (n, h)
                    outT_of[h_key] = (outT, h)
                    for g in range(NG):
                        score_unit(h_key, h, n, g)
                        # enqueue this head's AV work one group behind
                        for i in range(SG):
                            av_q.append((h_key, g * SG + i, n))
                        if n == 0 and idx == 0:
                            pop_filler(1)
                        else:
                            if n == 0 and idx == 1:
                                av_pop(2)
                                pop_filler(2)
                            else:
                                av_pop(4 if len(av_q) > 20 else 2)
                                pop_filler(1)
                        if idx == 1 and pending_proj:
                            pending_proj.pop(0)()
                pop_filler(len(fillers))
                if n == NQT - 1:
                    av_pop(len(av_q) + 2 * SG)
                    for u in make_proj_units(outT, n):
                        u()
                else:
                    pending_proj = make_proj_units(outT, n)

    nc.compile()
    return nc


def shard_inputs(x, W_qkv, W_out):
    """Full inputs -> list of 8 per-core input maps."""
    dt = ml_dtypes.bfloat16
    in_maps = []
    for c in range(N_CORES):
        b, g = divmod(c, 2)
        qcols = W_qkv[:, g * 256:(g + 1) * 256]
        kcols = W_qkv[:, INNER + g * 256:INNER + (g + 1) * 256]
        vcols = W_qkv[:, 2 * INNER + g * 256:2 * INNER + (g + 1) * 256]
        in_maps.append({
            "xT": np.ascontiguousarray(x[b].T).astype(dt),
            "wqk": np.ascontiguousarray(
                np.concatenate([qcols, kcols], axis=1)).astype(dt),
            "wv": np.ascontiguousarray(vcols).astype(dt),
            "wo": np.ascontiguousarray(
                W_out[g * 256:(g + 1) * 256, :]).astype(dt),
        })
    return in_maps


def gather_output(ys, b_out):
    out = np.empty((B, S, DO), np.float32)
    for b in range(B):
        out[b] = ys[2 * b] + ys[2 * b + 1]
        out[b] += b_out
    return out


_NC_CACHE = {}


def _get_nc():
    if "nc" not in _NC_CACHE:
        _NC_CACHE["nc"] = build_nc()
    return _NC_CACHE["nc"]


def kernel(**inputs):
    x = np.asarray(inputs["x"], np.float32)
    W_qkv = np.asarray(inputs["W_qkv"], np.float32)
    W_out = np.asarray(inputs["W_out"], np.float32)
    b_out = np.asarray(inputs["b_out"], np.float32)

    from concourse.bass_utils import run_bass_kernel_spmd

    nc = _get_nc()
    in_maps = shard_inputs(x, W_qkv, W_out)
    res = run_bass_kernel_spmd(nc, in_maps, core_ids=list(range(N_CORES)))
    ys = [r["y"] for r in res.results]
    return gather_output(ys, b_out)


# revision 24
# speedup vs baseline: 1.1472x; 1.1137x over previous
"""Multi-head attention (B=4, S=2048, D=512, H=8, DH=64) on 8 TRN2 NeuronCores.

Sharding: core c handles batch b = c//2 and head-group g = c%2 (4 of the 8
heads).  Each core computes its QKV projection (columns of W_qkv for its
heads), attention for its 4 heads, and a partial output projection
(rows of W_out for its heads).  The host sums the two partials per batch
and adds the bias.

v2 design notes (per-core; compute bf16, fp32 PSUM):
  - The Scalar (ACT) engine is a hard floor: exp over 16.8M attention
    scores at ~1 elem/cycle/partition @1.2GHz ~= 133us.  TensorE stream
    cycles total ~136us.  The schedule keeps both engines continuously
    busy; everything else (copies, normalize, DMAs) is pushed to
    DVE/GpSimd/sync which have slack.
  - q/k are stored unpadded, head-pairs packed across the 128 partitions
    (qk2[:, 0..3] = q01, q23, k01, k23; head h uses partitions
    (h%2)*64..+64).  Score matmuls contract over K=64.  No memset, half
    the copy volume of the padded layout.
  - Input DMAs are split and ordered (k01 weights + first x half first)
    so the first score matmuls issue ~2us in; the QKV projection work
    that is not needed immediately is woven as filler between tile-0
    score groups, so exp starts at ~6us instead of ~22us.
  - attn@V uses V in [token, dh+1] layout (ones column -> softmax
    denominator row in the same PSUM accumulation).  AV matmuls are fed
    from a work queue: each score group slot pops up to `cap` pending
    AV units (oldest head first), which self-balances the one-head-
    behind weave across tile boundaries and drains the backlog fast in
    the last tile.
  - normalize: reciprocal_approx_fast (5x cheaper than reciprocal, 18
    bits), partition-shift DMA + broadcast on the GpSimd queue (keeps
    the sync queue free for input/output DMAs), multiply on DVE.
  - out-projection of tile n runs as filler during tile n+1; tile 3's
    runs at the end (tail ~5us).
"""

import sys

for _p in ("/opt/trn_rl_repo", "/root/.axon_site/_ro/trn_rl_repo"):
    if _p not in sys.path:
        sys.path.append(_p)

import ml_dtypes
import numpy as np

import concourse.bass as bass
import concourse.tile as tile
from concourse import bacc, mybir

F32 = mybir.dt.float32
BF16 = mybir.dt.bfloat16
AF = mybir.ActivationFunctionType

# Problem dims (hardcoded per the grading contract).
B, S, D = 4, 2048, 512
H, DH = 8, 64
INNER = H * DH
HL = 4                # heads per core
DO = D                # output dim
QT = 512              # query tile
SCALE = DH ** -0.5

N_CORES = 8
CDT = BF16
# reciprocal_approx_fast (custom DVE ucode) returns garbage on this
# hardware/runtime (verified: CoreSim-correct, HW-NaN).  Exact reciprocal
# costs 3.3us per call on DVE but DVE has slack.
USE_APPROX_RECIP = False

KB = S // 128          # k-token blocks (16)
DC = D // 128          # contraction chunks for projections (4)
QKF = 2 * HL * DH      # q+k feature count per core (512)
MQK = QKF // 128       # qk feature blocks (4): m0=q01 m1=q23 m2=k01 m3=k23
VF = HL * DH           # v feature count (256)
NQT = S // QT          # query tiles (4)
SG = 2                 # exp supergroup: PSUM banks per ACTIVATE
NG = KB // SG          # score groups per head (8)


def build_nc(n_cores=N_CORES):
    nc = bacc.Bacc(
        "TRN2", target_bir_lowering=False, debug=False, num_devices=n_cores
    )
    xT = nc.dram_tensor("xT", [D, S], CDT, kind="ExternalInput").ap()
    wqk = nc.dram_tensor("wqk", [D, QKF], CDT, kind="ExternalInput").ap()
    wv = nc.dram_tensor("wv", [D, VF], CDT, kind="ExternalInput").ap()
    wo = nc.dram_tensor("wo", [VF, DO], CDT, kind="ExternalInput").ap()
    y = nc.dram_tensor("y", [S, DO], F32, kind="ExternalOutput").ap()

    with tile.TileContext(nc) as tc:
        with (
            tc.tile_pool(name="weights", bufs=1) as wpool,
            tc.tile_pool(name="big", bufs=1) as big,
            tc.tile_pool(name="psS", bufs=2, space="PSUM") as psS,
            tc.tile_pool(name="psB", bufs=2, space="PSUM") as psB,
            tc.tile_pool(name="psAV", bufs=2, space="PSUM") as psAV,
            tc.tile_pool(name="attnp", bufs=6) as attnp,
            tc.tile_pool(name="outp", bufs=2) as outp,
            tc.tile_pool(name="smalls", bufs=3) as smalls,
            tc.tile_pool(name="ysbp", bufs=3) as ysbp,
        ):
            # ---- input DMAs, ordered so k01 + q01/t0 compute starts asap ----
            wqk_sb = wpool.tile([128, DC, QKF], CDT)
            wqk_v = wqk.rearrange("(c p) f -> p c f", p=128)

            def wqk_dma(m):
                nc.sync.dma_start(
                    out=wqk_sb[:, :, m * 128:(m + 1) * 128],
                    in_=wqk_v[:, :, m * 128:(m + 1) * 128],
                )

            xT_sb = big.tile([128, DC, S], CDT)
            x_view = xT.rearrange("(c p) s -> c p s", p=128)

            def x_dma(c, half):
                sl = slice(half * (S // 2), (half + 1) * (S // 2))
                nc.sync.dma_start(out=xT_sb[:, c, sl], in_=x_view[c][:, sl])

            wv_sb = wpool.tile([128, DC, VF], CDT)
            wo_sb = wpool.tile([128, HL // 2, DO], CDT)

            wqk_dma(2)
            for c in range(DC):
                x_dma(c, 0)
            wqk_dma(0)
            nc.sync.dma_start(
                out=wv_sb, in_=wv.rearrange("(c p) f -> p c f", p=128)
            )
            for c in range(DC):
                x_dma(c, 1)
            wqk_dma(3)
            wqk_dma(1)
            nc.sync.dma_start(
                out=wo_sb, in_=wo.rearrange("(c p) d -> p c d", p=128)
            )

            # ---- persistent SBUF tensors ----
            # qkT chunk h    = qT of head h  (real rows (h%2)*64..+64, rest 0)
            # qkT chunk HL+h = kT of head h  (same padding).  The zero
            # padding keeps every matmul at K=128 (no mode-switch drains,
            # FWL stays enabled).  The pad memset is split per chunk (a
            # whole-tile memset is 13.7us on the DVE queue and blocks the
            # qk copies behind it).
            qkT = big.tile([128, 2 * HL, S], CDT)
            vaug = big.tile([128, KB, HL, DH + 1], CDT)
            ones_col = vaug[:, :, :, DH:DH + 1]
            nc.vector.memset(ones_col, 1.0)

            def ms_unit(c):
                rows = slice(64, 128) if c % 2 == 0 else slice(0, 64)
                nc.vector.memset(qkT[rows, c, :], 0.0)

            # ---- phase A work units (emitted inline or as phase-B filler) ----
            def qk_unit(m, t):
                # m: wqk feature block (m0=q01 m1=q23 m2=k01 m3=k23);
                # t: 512-token slice
                sl = slice(t * 512, (t + 1) * 512)
                base = HL if m >= 2 else 0
                hp = 2 * (m % 2)
                ps = psB.tile([128, 512], F32, tag="bank", name="psqk")
                for c in range(DC):
                    nc.tensor.matmul(
                        ps,
                        lhsT=wqk_sb[:, c, m * 128:(m + 1) * 128],
                        rhs=xT_sb[:, c, sl],
                        start=(c == 0),
                        stop=(c == DC - 1),
                    )
                nc.vector.tensor_copy(out=qkT[0:64, base + hp, sl],
                                      in_=ps[0:64, :])
                nc.vector.tensor_copy(out=qkT[64:128, base + hp + 1, sl],
                                      in_=ps[64:128, :])

            def v_unit(t):
                # t: 128-token block
                ps = psB.tile([128, 512], F32, tag="bank", name="psv")
                for c in range(DC):
                    nc.tensor.matmul(
                        ps[:, 0:VF],
                        lhsT=xT_sb[:, c, t * 128:(t + 1) * 128],
                        rhs=wv_sb[:, c, :],
                        start=(c == 0),
                        stop=(c == DC - 1),
                    )
                nc.vector.tensor_copy(
                    out=vaug[:, t, :, 0:DH],
                    in_=ps[:, 0:VF].rearrange("p (h e) -> p h e", h=HL),
                )

            # k01 for tokens 0-511 and q01 tile0 up front; everything else
            # becomes filler.
            ms_unit(5)
            ms_unit(1)
            qk_unit(2, 0)
            qk_unit(0, 0)
            ms_unit(4)
            ms_unit(0)
            # tile0-idx0 slots take 1 filler each (8), idx1 takes 2 (16),
            # idx2/idx3 take 1 each.  Emission order IS dependency order:
            # qk2x before the h1 score group reading that token slice;
            # v(kb) at least one slot before av(h1,kb) pops (idx1-g(kb//2));
            # k23/q23 chunks (qk30-33, qk10) + pads (ms7/ms3/ms6/ms2)
            # before h3 (idx2) / h2 (idx3) scores read them; qk0t/qk1t
            # before tile t.
            fillers = [
                ("qk", 2, 1), ("v", 0), ("qk", 2, 2), ("v", 1),
                ("qk", 2, 3), ("qk", 1, 0), ("ms", 7), ("ms", 3),
                ("v", 2), ("v", 3), ("v", 4), ("v", 5),
                ("v", 6), ("v", 7), ("v", 8), ("v", 9),
                ("v", 10), ("v", 11), ("v", 12), ("v", 13),
                ("v", 14), ("v", 15), ("qk", 3, 0), ("ms", 6),
                ("qk", 3, 1), ("ms", 2), ("qk", 3, 2), ("qk", 3, 3),
                ("qk", 0, 1), ("qk", 1, 1), ("qk", 0, 2), ("qk", 1, 2),
                ("qk", 0, 3), ("qk", 1, 3),
            ]

            def pop_filler(k):
                n = 0
                while fillers and n < k:
                    u = fillers.pop(0)
                    if u[0] == "qk":
                        qk_unit(u[1], u[2])
                    elif u[0] == "ms":
                        ms_unit(u[1])
                    else:
                        v_unit(u[1])
                    n += 1

            # ---- phase B: attention + output projection ----
            # AV work queue: entries (h_key, kb, tile_n); emitted oldest
            # first, up to `cap` per score-group slot.
            av_q = []
            avps = {}     # h_key -> psum tile
            outT_of = {}  # h_key -> (outT tile, h)

            def normalize(h_key):
                # Copy the whole AV psum to SBUF first: frees the avps ring
                # slot in ~0.7us instead of holding it through the recip
                # chain.  The reciprocal runs on the denominator reshaped to
                # [8, 64] via DMA (DVE op cost scales with free size only:
                # 0.42us vs 3.35us for [1, 512]).
                outT, h = outT_of[h_key]
                ps = avps.pop(h_key)
                o_un = smalls.tile([DH + 1, QT], F32, tag="oun")
                nc.vector.tensor_copy(out=o_un, in_=ps)
                rdq = smalls.tile([8, QT // 8], F32, tag="rdq")
                nc.sync.dma_start(out=rdq, in_=o_un[DH:DH + 1, :])
                rdr = smalls.tile([8, QT // 8], F32, tag="rdr")
                nc.vector.reciprocal(rdr, rdq)
                rd0 = smalls.tile([1, QT], F32, tag="rd0")
                nc.sync.dma_start(out=rd0, in_=rdr)
                rb = smalls.tile([64, QT], F32, tag="rb")
                nc.gpsimd.partition_broadcast(rb, rd0, channels=64)
                if h % 2 == 0:
                    nc.vector.tensor_mul(
                        outT[0:64, h // 2, :], o_un[0:DH, :], rb
                    )
                else:
                    ot = smalls.tile([64, QT], CDT, tag="ot")
                    nc.vector.tensor_mul(ot, o_un[0:DH, :], rb)
                    nc.sync.dma_start(out=outT[64:128, h // 2, :], in_=ot)

            def av_pop(cap):
                n = 0
                while av_q and n < cap:
                    h_key, kb, tn = av_q.pop(0)
                    if kb == 0:
                        avps[h_key] = psAV.tile(
                            [DH + 1, QT], F32, tag="avp", name="avp"
                        )
                    _, h = outT_of[h_key]
                    at = at_of[h_key]
                    nc.tensor.matmul(
                        avps[h_key],
                        lhsT=vaug[:, kb, h, :],
                        rhs=at[:, kb, :],
                        start=(kb == 0),
                        stop=(kb == KB - 1),
                        skip_group_check=True,
                    )
                    n += 1
                    if kb == KB - 1:
                        normalize(h_key)

            at_of = {}

            def score_unit(h_key, h, n, g):
                if g == 0:
                    at_of[h_key] = attnp.tile(
                        [128, KB, QT], CDT, tag="attnT", name="at"
                    )
                qs = qkT[:, h, n * QT:(n + 1) * QT]
                ps = psS.tile([128, SG, 512], F32, tag="sc")
                for i in range(SG):
                    kb = g * SG + i
                    nc.tensor.matmul(
                        ps[:, i, :],
                        lhsT=qkT[:, HL + h, kb * 128:(kb + 1) * 128],
                        rhs=qs,
                        skip_group_check=True,
                    )
                nc.scalar.activation(
                    out=at_of[h_key][:, g * SG:(g + 1) * SG, :], in_=ps,
                    func=AF.Exp, scale=SCALE,
                )

            pending_proj = []

            def make_proj_units(outT, n):
                units = []
                for qb in range(QT // 128):
                    def unit(qb=qb, outT=outT, n=n):
                        yps = psB.tile([128, DO], F32, tag="bank", name="yps")
                        for c in range(HL // 2):
                            nc.tensor.matmul(
                                yps,
                                lhsT=outT[:, c, qb * 128:(qb + 1) * 128],
                                rhs=wo_sb[:, c, :],
                                start=(c == 0),
                                stop=(c == HL // 2 - 1),
                                skip_group_check=True,
                            )
                        ysb = ysbp.tile([128, DO], F32, tag="ysb")
                        nc.vector.tensor_copy(out=ysb, in_=yps)
                        nc.sync.dma_start(
                            out=y[n * QT + qb * 128:
                                  n * QT + (qb + 1) * 128, :],
                            in_=ysb,
                        )
                    units.append(unit)
                return units

            for n in range(NQT):
                outT = outp.tile([128, HL // 2, QT], CDT, tag="outT")
                order = (1, 0, 3, 2) if n == 0 else (1, 3, 0, 2)
                for idx, h in enumerate(order):
                    h_key = (n, h)
                    outT_of[h_key] = (outT, h)
                    for g in range(NG):
                        score_unit(h_key, h, n, g)
                        # enqueue this head's AV work one group behind
                        for i in range(SG):
                            av_q.append((h_key, g * SG + i, n))
                        if n == 0 and idx == 0:
                            pop_filler(1)
                        else:
                            if n == 0 and idx == 1:
                                av_pop(2)
                                pop_filler(2)
                            else:
                                thresh = 6 if n == NQT - 1 else 20
                                av_pop(4 if len(av_q) > thresh else 2)
                                pop_filler(1)
                        if idx == 1 and pending_proj:
                            pending_proj.pop(0)()
                pop_filler(len(fillers))
                if n == NQT - 1:
                    av_pop(len(av_q) + 2 * SG)
                    for u in make_proj_units(outT, n):
                        u()
                else:
                    pending_proj = make_proj_units(outT, n)

    nc.compile()
    return nc


def shard_inputs(x, W_qkv, W_out):
    """Full inputs -> list of 8 per-core input maps."""
    dt = ml_dtypes.bfloat16
    in_maps = []
    for c in range(N_CORES):
        b, g = divmod(c, 2)
        qcols = W_qkv[:, g * 256:(g + 1) * 256]
        kcols = W_qkv[:, INNER + g * 256:INNER + (g + 1) * 256]
        vcols = W_qkv[:, 2 * INNER + g * 256:2 * INNER + (g + 1) * 256]
        in_maps.append({
            "xT": np.ascontiguousarray(x[b].T).astype(dt),
            "wqk": np.ascontiguousarray(
                np.concatenate([qcols, kcols], axis=1)).astype(dt),
            "wv": np.ascontiguousarray(vcols).astype(dt),
            "wo": np.ascontiguousarray(
                W_out[g * 256:(g + 1) * 256, :]).astype(dt),
        })
    return in_maps


def gather_output(ys, b_out):
    out = np.empty((B, S, DO), np.float32)
    for b in range(B):
        out[b] = ys[2 * b] + ys[2 * b + 1]
        out[b] += b_out
    return out


_NC_CACHE = {}


def _get_nc():
    if "nc" not in _NC_CACHE:
        _NC_CACHE["nc"] = build_nc()
    return _NC_CACHE["nc"]


def kernel(**inputs):
    x = np.asarray(inputs["x"], np.float32)
    W_qkv = np.asarray(inputs["W_qkv"], np.float32)
    W_out = np.asarray(inputs["W_out"], np.float32)
    b_out = np.asarray(inputs["b_out"], np.float32)

    from concourse.bass_utils import run_bass_kernel_spmd

    nc = _get_nc()
    in_maps = shard_inputs(x, W_qkv, W_out)
    res = run_bass_kernel_spmd(nc, in_maps, core_ids=list(range(N_CORES)))
    ys = [r["y"] for r in res.results]
    return gather_output(ys, b_out)


# revision 29
# speedup vs baseline: 1.2191x; 1.0627x over previous
"""Multi-head attention (B=4, S=2048, D=512, H=8, DH=64) on 8 TRN2 NeuronCores.

Sharding: core c handles batch b = c//2 and head-group g = c%2 (4 of the 8
heads).  Each core computes its QKV projection (columns of W_qkv for its
heads), attention for its 4 heads, and a partial output projection
(rows of W_out for its heads).  The host sums the two partials per batch
and adds the bias.

v2 design notes (per-core; compute bf16, fp32 PSUM):
  - The Scalar (ACT) engine is a hard floor: exp over 16.8M attention
    scores at ~1 elem/cycle/partition @1.2GHz ~= 133us.  TensorE stream
    cycles total ~136us.  The schedule keeps both engines continuously
    busy; everything else (copies, normalize, DMAs) is pushed to
    DVE/GpSimd/sync which have slack.
  - q/k are stored unpadded, head-pairs packed across the 128 partitions
    (qk2[:, 0..3] = q01, q23, k01, k23; head h uses partitions
    (h%2)*64..+64).  Score matmuls contract over K=64.  No memset, half
    the copy volume of the padded layout.
  - Input DMAs are split and ordered (k01 weights + first x half first)
    so the first score matmuls issue ~2us in; the QKV projection work
    that is not needed immediately is woven as filler between tile-0
    score groups, so exp starts at ~6us instead of ~22us.
  - attn@V uses V in [token, dh+1] layout (ones column -> softmax
    denominator row in the same PSUM accumulation).  AV matmuls are fed
    from a work queue: each score group slot pops up to `cap` pending
    AV units (oldest head first), which self-balances the one-head-
    behind weave across tile boundaries and drains the backlog fast in
    the last tile.
  - normalize: reciprocal_approx_fast (5x cheaper than reciprocal, 18
    bits), partition-shift DMA + broadcast on the GpSimd queue (keeps
    the sync queue free for input/output DMAs), multiply on DVE.
  - out-projection of tile n runs as filler during tile n+1; tile 3's
    runs at the end (tail ~5us).
"""

import sys

for _p in ("/opt/trn_rl_repo", "/root/.axon_site/_ro/trn_rl_repo"):
    if _p not in sys.path:
        sys.path.append(_p)

import ml_dtypes
import numpy as np

import concourse.bass as bass
import concourse.tile as tile
from concourse import bacc, mybir

F32 = mybir.dt.float32
BF16 = mybir.dt.bfloat16
AF = mybir.ActivationFunctionType

# Problem dims (hardcoded per the grading contract).
B, S, D = 4, 2048, 512
H, DH = 8, 64
INNER = H * DH
HL = 4                # heads per core
DO = D                # output dim
QT = 512              # query tile
SCALE = DH ** -0.5

N_CORES = 8
CDT = BF16
# reciprocal_approx_fast (custom DVE ucode) returns garbage on this
# hardware/runtime (verified: CoreSim-correct, HW-NaN).  Exact reciprocal
# costs 3.3us per call on DVE but DVE has slack.
USE_APPROX_RECIP = False

KB = S // 128          # k-token blocks (16)
DC = D // 128          # contraction chunks for projections (4)
QKF = 2 * HL * DH      # q+k feature count per core (512)
MQK = QKF // 128       # qk feature blocks (4): m0=q01 m1=q23 m2=k01 m3=k23
VF = HL * DH           # v feature count (256)
NQT = S // QT          # query tiles (4)
SG = 2                 # exp supergroup: PSUM banks per ACTIVATE
NG = KB // SG          # score groups per head (8)


def build_nc(n_cores=N_CORES):
    nc = bacc.Bacc(
        "TRN2", target_bir_lowering=False, debug=False, num_devices=n_cores
    )
    xT = nc.dram_tensor("xT", [D, S], CDT, kind="ExternalInput").ap()
    wqk = nc.dram_tensor("wqk", [D, QKF], CDT, kind="ExternalInput").ap()
    wv = nc.dram_tensor("wv", [D, VF], CDT, kind="ExternalInput").ap()
    wo = nc.dram_tensor("wo", [VF, DO], CDT, kind="ExternalInput").ap()
    y = nc.dram_tensor("y", [S, DO], F32, kind="ExternalOutput").ap()

    with tile.TileContext(nc) as tc:
        with (
            tc.tile_pool(name="weights", bufs=1) as wpool,
            tc.tile_pool(name="big", bufs=1) as big,
            tc.tile_pool(name="psS", bufs=2, space="PSUM") as psS,
            tc.tile_pool(name="psB", bufs=2, space="PSUM") as psB,
            tc.tile_pool(name="psAV", bufs=2, space="PSUM") as psAV,
            tc.tile_pool(name="attnp", bufs=6) as attnp,
            tc.tile_pool(name="outp", bufs=2) as outp,
            tc.tile_pool(name="smalls", bufs=3) as smalls,
            tc.tile_pool(name="ysbp", bufs=3) as ysbp,
        ):
            # ---- input DMAs, ordered so k01 + q01/t0 compute starts asap ----
            wqk_sb = wpool.tile([128, DC, QKF], CDT)
            wqk_v = wqk.rearrange("(c p) f -> p c f", p=128)

            def wqk_dma(m):
                nc.sync.dma_start(
                    out=wqk_sb[:, :, m * 128:(m + 1) * 128],
                    in_=wqk_v[:, :, m * 128:(m + 1) * 128],
                )

            xT_sb = big.tile([128, DC, S], CDT)
            x_view = xT.rearrange("(c p) s -> c p s", p=128)

            def x_dma(c, half):
                sl = slice(half * (S // 2), (half + 1) * (S // 2))
                nc.sync.dma_start(out=xT_sb[:, c, sl], in_=x_view[c][:, sl])

            wv_sb = wpool.tile([128, DC, VF], CDT)
            wo_sb = wpool.tile([128, HL // 2, DO], CDT)

            wqk_dma(2)
            for c in range(DC):
                x_dma(c, 0)
            wqk_dma(0)
            nc.sync.dma_start(
                out=wv_sb, in_=wv.rearrange("(c p) f -> p c f", p=128)
            )
            for cp in range(DC // 2):
                sl = slice(S // 2, S)
                nc.sync.dma_start(
                    out=xT_sb[:, 2 * cp:2 * cp + 2, sl],
                    in_=x_view[2 * cp:2 * cp + 2].rearrange("c p s -> p c s")[
                        :, :, sl],
                )
            wqk_dma(3)
            wqk_dma(1)
            nc.sync.dma_start(
                out=wo_sb, in_=wo.rearrange("(c p) d -> p c d", p=128)
            )

            # ---- persistent SBUF tensors ----
            # qkT chunk h    = qT of head h  (real rows (h%2)*64..+64, rest 0)
            # qkT chunk HL+h = kT of head h  (same padding).  The zero
            # padding keeps every matmul at K=128 (no mode-switch drains,
            # FWL stays enabled).  The pad memset is split per chunk (a
            # whole-tile memset is 13.7us on the DVE queue and blocks the
            # qk copies behind it).
            qkT = big.tile([128, 2 * HL, S], CDT)
            vaug = big.tile([128, KB, HL, DH + 1], CDT)
            ones_col = vaug[:, :, :, DH:DH + 1]
            nc.gpsimd.memset(ones_col, 1.0)

            def ms_unit(c, eng=None):
                rows = slice(64, 128) if c % 2 == 0 else slice(0, 64)
                (eng or nc.gpsimd).memset(qkT[rows, c, :], 0.0)

            # ---- phase A work units (emitted inline or as phase-B filler) ----
            def qk_unit(m, t):
                # m: wqk feature block (m0=q01 m1=q23 m2=k01 m3=k23);
                # t: 512-token slice
                sl = slice(t * 512, (t + 1) * 512)
                base = HL if m >= 2 else 0
                hp = 2 * (m % 2)
                ps = psB.tile([128, 512], F32, tag="bank", name="psqk")
                for c in range(DC):
                    nc.tensor.matmul(
                        ps,
                        lhsT=wqk_sb[:, c, m * 128:(m + 1) * 128],
                        rhs=xT_sb[:, c, sl],
                        start=(c == 0),
                        stop=(c == DC - 1),
                    )
                nc.vector.tensor_copy(out=qkT[0:64, base + hp, sl],
                                      in_=ps[0:64, :])
                nc.vector.tensor_copy(out=qkT[64:128, base + hp + 1, sl],
                                      in_=ps[64:128, :])

            def v_unit(t):
                # t: 128-token block
                ps = psB.tile([128, 512], F32, tag="bank", name="psv")
                for c in range(DC):
                    nc.tensor.matmul(
                        ps[:, 0:VF],
                        lhsT=xT_sb[:, c, t * 128:(t + 1) * 128],
                        rhs=wv_sb[:, c, :],
                        start=(c == 0),
                        stop=(c == DC - 1),
                    )
                nc.vector.tensor_copy(
                    out=vaug[:, t, :, 0:DH],
                    in_=ps[:, 0:VF].rearrange("p (h e) -> p h e", h=HL),
                )

            # k01 for tokens 0-511 and q01 tile0 up front; everything else
            # becomes filler.  The two startup-critical pad memsets run on
            # DVE (fast, before the copies need the chunks); the rest run
            # on the otherwise-idle GpSimd engine.
            ms_unit(5, nc.vector)
            ms_unit(1, nc.vector)
            for c in (4, 0, 7, 3, 6, 2):
                ms_unit(c)
            qk_unit(2, 0)
            qk_unit(0, 0)
            # tile0-idx0 slots take 1 filler each (8), idx1 takes 2 (16),
            # idx2/idx3 take 1 each.  Emission order IS dependency order:
            # qk2x before the h1 score group reading that token slice;
            # v(kb) at least one slot before av(h1,kb) pops (idx1-g(kb//2));
            # k23/q23 chunks (qk30-33, qk10) before h3 (idx2) / h2 (idx3)
            # scores read them; qk0t/qk1t before tile t.
            fillers = [
                ("qk", 2, 1), ("v", 0), ("qk", 2, 2), ("v", 1),
                ("qk", 2, 3), ("qk", 1, 0),
                ("v", 2), ("v", 3), ("v", 4), ("v", 5),
                ("v", 6), ("v", 7), ("v", 8), ("v", 9),
                ("v", 10), ("v", 11), ("v", 12), ("v", 13),
                ("v", 14), ("v", 15), ("qk", 3, 0),
                ("qk", 3, 1), ("qk", 3, 2), ("qk", 3, 3),
                ("qk", 0, 1), ("qk", 1, 1), ("qk", 0, 2), ("qk", 1, 2),
                ("qk", 0, 3), ("qk", 1, 3),
            ]

            def pop_filler(k):
                n = 0
                while fillers and n < k:
                    u = fillers.pop(0)
                    if u[0] == "qk":
                        qk_unit(u[1], u[2])
                    elif u[0] == "ms":
                        ms_unit(u[1])
                    else:
                        v_unit(u[1])
                    n += 1

            # ---- phase B: attention + output projection ----
            # AV work queue: entries (h_key, kb, tile_n); emitted oldest
            # first, up to `cap` per score-group slot.
            av_q = []
            avps = {}     # h_key -> psum tile
            outT_of = {}  # h_key -> (outT tile, h)

            def normalize(h_key):
                # Copy the whole AV psum to SBUF first: frees the avps ring
                # slot in ~0.7us instead of holding it through the recip
                # chain.  The reciprocal runs on the denominator reshaped to
                # [8, 64] via DMA (DVE op cost scales with free size only:
                # 0.42us vs 3.35us for [1, 512]).
                outT, h = outT_of[h_key]
                ps = avps.pop(h_key)
                o_un = smalls.tile([DH + 1, QT], F32, tag="oun")
                nc.vector.tensor_copy(out=o_un, in_=ps)
                rdq = smalls.tile([8, QT // 8], F32, tag="rdq")
                nc.sync.dma_start(out=rdq, in_=o_un[DH:DH + 1, :])
                rdr = smalls.tile([8, QT // 8], F32, tag="rdr")
                nc.vector.reciprocal(rdr, rdq)
                rd0 = smalls.tile([1, QT], F32, tag="rd0")
                nc.sync.dma_start(out=rd0, in_=rdr)
                rb = smalls.tile([64, QT], F32, tag="rb")
                nc.gpsimd.partition_broadcast(rb, rd0, channels=64)
                if h % 2 == 0:
                    nc.vector.tensor_mul(
                        outT[0:64, h // 2, :], o_un[0:DH, :], rb
                    )
                else:
                    ot = smalls.tile([64, QT], CDT, tag="ot")
                    nc.vector.tensor_mul(ot, o_un[0:DH, :], rb)
                    nc.sync.dma_start(out=outT[64:128, h // 2, :], in_=ot)

            FINAL_KEY = (NQT - 1, 2)  # last head of the last tile's order

            def normalize_final(h_key):
                # Fused tail: per-128q normalize multiply feeding its
                # projection unit immediately, so proj/ysb/y-DMA pipeline
                # against the remaining normalize chunks.
                outT, h = outT_of[h_key]
                ps = avps.pop(h_key)
                den = smalls.tile([1, QT], F32, tag="denf")
                nc.vector.tensor_copy(out=den, in_=ps[DH:DH + 1, :])
                rdq = smalls.tile([8, QT // 8], F32, tag="rdq")
                nc.sync.dma_start(out=rdq, in_=den)
                rdr = smalls.tile([8, QT // 8], F32, tag="rdr")
                nc.vector.reciprocal(rdr, rdq)
                rd0 = smalls.tile([1, QT], F32, tag="rd0")
                nc.sync.dma_start(out=rd0, in_=rdr)
                rb = smalls.tile([64, QT], F32, tag="rb")
                nc.gpsimd.partition_broadcast(rb, rd0, channels=64)
                units = make_proj_units(outT, h_key[0])
                for qb in range(QT // 128):
                    sl = slice(qb * 128, (qb + 1) * 128)
                    nc.vector.tensor_mul(
                        outT[0:64, h // 2, sl], ps[0:DH, sl], rb[:, sl]
                    )
                    units[qb]()

            def av_pop(cap):
                n = 0
                while av_q and n < cap:
                    h_key, kb, tn = av_q.pop(0)
                    if kb == 0:
                        avps[h_key] = psAV.tile(
                            [DH + 1, QT], F32, tag="avp", name="avp"
                        )
                    _, h = outT_of[h_key]
                    at = at_of[h_key]
                    nc.tensor.matmul(
                        avps[h_key],
                        lhsT=vaug[:, kb, h, :],
                        rhs=at[:, kb, :],
                        start=(kb == 0),
                        stop=(kb == KB - 1),
                        skip_group_check=True,
                    )
                    n += 1
                    if kb == KB - 1:
                        if h_key == FINAL_KEY:
                            normalize_final(h_key)
                        else:
                            normalize(h_key)

            at_of = {}

            def score_unit(h_key, h, n, g):
                if g == 0:
                    at_of[h_key] = attnp.tile(
                        [128, KB, QT], CDT, tag="attnT", name="at"
                    )
                qs = qkT[:, h, n * QT:(n + 1) * QT]
                ps = psS.tile([128, SG, 512], F32, tag="sc")
                for i in range(SG):
                    kb = g * SG + i
                    nc.tensor.matmul(
                        ps[:, i, :],
                        lhsT=qkT[:, HL + h, kb * 128:(kb + 1) * 128],
                        rhs=qs,
                        skip_group_check=True,
                    )
                nc.scalar.activation(
                    out=at_of[h_key][:, g * SG:(g + 1) * SG, :], in_=ps,
                    func=AF.Exp, scale=SCALE,
                )

            pending_proj = []

            def make_proj_units(outT, n):
                units = []
                for qb in range(QT // 128):
                    def unit(qb=qb, outT=outT, n=n):
                        yps = psB.tile([128, DO], F32, tag="bank", name="yps")
                        for c in range(HL // 2):
                            nc.tensor.matmul(
                                yps,
                                lhsT=outT[:, c, qb * 128:(qb + 1) * 128],
                                rhs=wo_sb[:, c, :],
                                start=(c == 0),
                                stop=(c == HL // 2 - 1),
                                skip_group_check=True,
                            )
                        ysb = ysbp.tile([128, DO], F32, tag="ysb")
                        nc.vector.tensor_copy(out=ysb, in_=yps)
                        nc.sync.dma_start(
                            out=y[n * QT + qb * 128:
                                  n * QT + (qb + 1) * 128, :],
                            in_=ysb,
                        )
                    units.append(unit)
                return units

            for n in range(NQT):
                outT = outp.tile([128, HL // 2, QT], CDT, tag="outT")
                order = (1, 0, 3, 2) if n == 0 else (1, 3, 0, 2)
                for idx, h in enumerate(order):
                    h_key = (n, h)
                    outT_of[h_key] = (outT, h)
                    for g in range(NG):
                        score_unit(h_key, h, n, g)
                        # enqueue this head's AV work one group behind
                        for i in range(SG):
                            av_q.append((h_key, g * SG + i, n))
                        if n == 0 and idx == 0:
                            pop_filler(1)
                        else:
                            if n == 0 and idx == 1:
                                av_pop(2)
                                pop_filler(2)
                            else:
                                thresh = 6 if n == NQT - 1 else 20
                                av_pop(4 if len(av_q) > thresh else 2)
                                pop_filler(1)
                        if idx == 1 and pending_proj:
                            pending_proj.pop(0)()
                pop_filler(len(fillers))
                if n == NQT - 1:
                    # drain: the FINAL_KEY kb15 pop emits the fused
                    # normalize+projection tail.
                    av_pop(len(av_q) + 2 * SG)
                else:
                    pending_proj = make_proj_units(outT, n)

    nc.compile()
    return nc


def shard_inputs(x, W_qkv, W_out):
    """Full inputs -> list of 8 per-core input maps."""
    dt = ml_dtypes.bfloat16
    in_maps = []
    for c in range(N_CORES):
        b, g = divmod(c, 2)
        qcols = W_qkv[:, g * 256:(g + 1) * 256]
        kcols = W_qkv[:, INNER + g * 256:INNER + (g + 1) * 256]
        vcols = W_qkv[:, 2 * INNER + g * 256:2 * INNER + (g + 1) * 256]
        in_maps.append({
            "xT": np.ascontiguousarray(x[b].T).astype(dt),
            "wqk": np.ascontiguousarray(
                np.concatenate([qcols, kcols], axis=1)).astype(dt),
            "wv": np.ascontiguousarray(vcols).astype(dt),
            "wo": np.ascontiguousarray(
                W_out[g * 256:(g + 1) * 256, :]).astype(dt),
        })
    return in_maps


def gather_output(ys, b_out):
    out = np.empty((B, S, DO), np.float32)
    for b in range(B):
        out[b] = ys[2 * b] + ys[2 * b + 1]
        out[b] += b_out
    return out


_NC_CACHE = {}


def _get_nc():
    if "nc" not in _NC_CACHE:
        _NC_CACHE["nc"] = build_nc()
    return _NC_CACHE["nc"]


def kernel(**inputs):
    x = np.asarray(inputs["x"], np.float32)
    W_qkv = np.asarray(inputs["W_qkv"], np.float32)
    W_out = np.asarray(inputs["W_out"], np.float32)
    b_out = np.asarray(inputs["b_out"], np.float32)

    from concourse.bass_utils import run_bass_kernel_spmd

    nc = _get_nc()
    in_maps = shard_inputs(x, W_qkv, W_out)
    res = run_bass_kernel_spmd(nc, in_maps, core_ids=list(range(N_CORES)))
    ys = [r["y"] for r in res.results]
    return gather_output(ys, b_out)
